# revision 16
# baseline (speedup 1.0000x reference)
"""Trainium2 Bass kernel for nn_MixedFeedFoward (DARTS-style mixed-architecture MLP).

Math: out = relu(x @ (m0*w0).T + bm0*b0) @ (m1*w1).T + bm1*b1
The DARTS masks are rank-structured.  With a = softmax(arch_embed),
b = softmax(arch_mlp), EMBED = (512,768,1024), RATIO = (2,3,4):

  s_e[h]     = sum_r b_r * [h < e*r]
  g_j[h]     = sum_{e_idx >= j} a_e * s_e[h]
  c_j        = sum_{e_idx >= j} a_e
  W0eff[h,d] = w0[h,d] * g_{blk(d)}[h]      blk(d): 0 for d<512, 1 for d<768, else 2
  bm0[h]     = g_0[h]
  W1eff[d,h] = w1[d,h] * g_{blk(d)}[h]
  bm1[d]     = c_{blk(d)}

g_j is constant on 256-aligned h segments, so masking reduces to 51 runtime
scalars (3 j's x 16 segments + 3 c's), computed on device from the arch inputs
via one tiny matmul against a constant 0/1 selection table.

Sharding: data-parallel over the 4096 tokens -> 512 tokens per core; every core
streams the full weights (cast fp32->bf16 in the DMA engines).  Layer 0
computes hT [H, T] (weights stationary, tokens moving); layer 1 consumes hT
per 512-row h-group, accumulating outT [D, T] partials in PSUM per group and
summing groups into an SBUF fp32 accumulator.
"""

import os

import numpy as np

import concourse.bass as bass
import concourse.mybir as mybir
from concourse import bacc
from concourse.bass_utils import run_bass_kernel_spmd
from concourse.tile import TileContext

N_CORES = 8
D = 1024          # embed dim
H = 4096          # expansion dim
T = 512           # tokens per core (4096 total / 8 cores)
P = 128
SEG = 256         # h-segment size on which g_j is constant
NSEG = H // SEG   # 16
EMBED = (512, 768, 1024)
RATIO = (2, 3, 4)

F32 = mybir.dt.float32
BF16 = mybir.dt.bfloat16
AF = mybir.ActivationFunctionType
ALU = mybir.AluOpType

MM_DTYPE = os.environ.get("BASS_MM_DTYPE", "bf16")


def _build_k2() -> np.ndarray:
    """Constant 0/1 selection table: G_flat[col] = sum_i P9[i] * K2[i, col]
    where P9[e*3+r] = a_e * b_r.
    cols 0..47: col = j*16 + seg -> [e_idx >= j] * [seg*SEG < e*r]
    cols 48..50: col = 48 + j   -> [e_idx >= j]   (since sum_r b_r = 1)
    """
    k2 = np.zeros((9, 51), dtype=np.float32)
    for ie, e in enumerate(EMBED):
        for ir, r in enumerate(RATIO):
            i = ie * 3 + ir
            for j in range(3):
                if ie >= j:
                    for seg in range(NSEG):
                        if seg * SEG < e * r:
                            k2[i, j * 16 + seg] = 1.0
                    k2[i, 48 + j] = 1.0
    return k2


_K2 = _build_k2()

# d-block of each 128-wide d-chunk (0..7): [0,512)->0, [512,768)->1, [768,1024)->2
_DBLK = [0, 0, 0, 0, 1, 1, 2, 2]


def _build_nc() -> bass.Bass:
    nc = bacc.Bacc("TRN2", target_bir_lowering=False, debug=False)

    xT_d = nc.dram_tensor("xT", [D, T], F32, kind="ExternalInput")
    w0T_d = nc.dram_tensor("w0T", [D, H], F32, kind="ExternalInput")
    w1T_d = nc.dram_tensor("w1T", [H, D], F32, kind="ExternalInput")
    b0r_d = nc.dram_tensor("b0r", [P, H // P], F32, kind="ExternalInput")
    b1r_d = nc.dram_tensor("b1r", [P, D // P], F32, kind="ExternalInput")
    ae9_d = nc.dram_tensor("ae9", [9, 1], F32, kind="ExternalInput")
    am9_d = nc.dram_tensor("am9", [9, 1], F32, kind="ExternalInput")
    k2_d = nc.dram_tensor("k2", [9, 51], F32, kind="ExternalInput")
    out_d = nc.dram_tensor("outT", [D, T], F32, kind="ExternalOutput")
    gsc_d = nc.dram_tensor("gscratch", [51], F32)

    mm_dt = BF16 if MM_DTYPE == "bf16" else F32

    def mm_ap(ap):
        return ap.bitcast(mybir.dt.float32r) if MM_DTYPE == "f32r" else ap

    xdma = nc.gpsimd if mm_dt == BF16 else nc.sync  # cast-DMA needs SWDGE

    with TileContext(nc) as tc:
        with (
            tc.tile_pool(name="const", bufs=1) as const,
            tc.tile_pool(name="w0f", bufs=8) as w0f_pool,
            tc.tile_pool(name="xfp", bufs=3) as xf_pool,
            tc.tile_pool(name="w0p", bufs=14) as w0_pool,
            tc.tile_pool(name="w1f", bufs=6) as w1f_pool,
            tc.tile_pool(name="w1p", bufs=6) as w1_pool,
            tc.tile_pool(name="ps0", bufs=3, space="PSUM") as ps0_pool,
            tc.tile_pool(name="ps1", bufs=3, space="PSUM") as ps1_pool,
            tc.tile_pool(name="psp", bufs=1, space="PSUM") as psp_pool,
        ):
            # ---------------- arch-weight prep (tiny) ----------------
            # E9[e*3+r] = exp(ae[e] + am[r]);  sum(E9) = sum_e exp(ae)*sum_r exp(am)
            ae9 = const.tile([9, 1], F32, tag="ae9")
            am9 = const.tile([9, 1], F32, tag="am9")
            nc.gpsimd.dma_start(ae9[:], ae9_d[:, :])
            nc.gpsimd.dma_start(am9[:], am9_d[:, :])
            v9 = const.tile([9, 1], F32, tag="v9")
            nc.vector.tensor_tensor(v9[:], ae9[:], am9[:], ALU.add)
            e9 = const.tile([9, 1], F32, tag="e9")
            nc.scalar.activation(e9[:], v9[:], AF.Exp)
            ones9 = const.tile([9, 1], F32, tag="ones9")
            nc.vector.memset(ones9[:], 1.0)
            s_ps = psp_pool.tile([128, 51], F32, tag="psp", name="s_ps")[0:1, 0:1]
            nc.tensor.matmul(s_ps[:], ones9[:], e9[:], start=True, stop=True)
            s_sb = const.tile([1, 1], F32, tag="s_sb")
            nc.vector.tensor_copy(s_sb[:], s_ps[:])
            rec = const.tile([1, 1], F32, tag="rec")
            nc.vector.reciprocal(rec[:], s_sb[:])

            k2_sb = const.tile([9, 51], F32, tag="k2_sb")
            nc.gpsimd.dma_start(k2_sb[:], k2_d[:, :])
            g_ps = psp_pool.tile([128, 51], F32, tag="psp", name="g_ps")[0:1, :]
            nc.tensor.matmul(g_ps[:], e9[:], k2_sb[:], start=True, stop=True)
            g_row = const.tile([1, 51], F32, tag="g_row")
            nc.vector.tensor_scalar(g_row[:], g_ps[:], rec[0:1, 0:1], None, ALU.mult)
            # broadcast to all 128 partitions via a k=1 ones-matmul
            ones128 = const.tile([1, P], F32, tag="ones128")
            nc.vector.memset(ones128[:], 1.0)
            gb_ps = psp_pool.tile([128, 51], F32, tag="psp", name="gb_ps")
            nc.tensor.matmul(gb_ps[:], ones128[:], g_row[:], start=True, stop=True)
            gb = const.tile([P, 51], F32, tag="gb")
            nc.vector.tensor_copy(gb[:], gb_ps[:])

            # ---------------- effective biases ----------------
            b0_sb = const.tile([P, H // P], F32, tag="b0_sb")
            nc.gpsimd.dma_start(b0_sb[:], b0r_d[:, :])
            bb0 = const.tile([P, H // P], F32, tag="bb0")
            nc.vector.tensor_tensor(
                bb0[:].rearrange("p (s i) -> p s i", i=2),
                b0_sb[:].rearrange("p (s i) -> p s i", i=2),
                gb[:, 0:16].unsqueeze(2).to_broadcast((P, 16, 2)),
                ALU.mult,
            )
            b1_sb = const.tile([P, D // P], F32, tag="b1_sb")
            nc.gpsimd.dma_start(b1_sb[:], b1r_d[:, :])
            bb1 = const.tile([P, D // P], F32, tag="bb1")
            for j, (c0, c1) in enumerate([(0, 4), (4, 6), (6, 8)]):
                nc.vector.tensor_scalar(
                    bb1[:, c0:c1], b1_sb[:, c0:c1],
                    gb[:, 48 + j : 49 + j], None, ALU.mult,
                )

            # ---------------- PE warmup ----------------
            # Keep the PE busy from ~t+4us so the HAM clock gate reaches
            # K=8/8 before the first real matmuls and stays warm through
            # the initial weight-DMA ramp (otherwise the first ~20us of
            # matmuls run at 1.2 GHz).
            junk_w = const.tile([P, 2 * P], mm_dt, tag="junk_w")
            nc.vector.memset(junk_w[:], 0.0)
            junk_x = const.tile([P, T], mm_dt, tag="junk_x")
            nc.vector.memset(junk_x[:], 0.0)
            ps_w = psp_pool.tile([P, T], F32, tag="warm", name="ps_w")
            NWARM = 20
            for i in range(NWARM):
                # alternate weight slices so LDWEIGHTS prefetches into the
                # background buffer and the PE sees full-duty matmul activity
                sl = (i % 2) * P
                nc.tensor.matmul(
                    ps_w[:], mm_ap(junk_w[:, sl : sl + P]), mm_ap(junk_x[:]),
                    start=(i == 0), stop=(i == NWARM - 1),
                )

            # ---------------- xT load (HWDGE fp32 + DVE cast) ----------------
            # x gates every L0 chain; keep it on the fast HWDGE queue, first.
            xt_sb = []
            for k in range(D // P):
                xf = xf_pool.tile([P, T], F32, tag="xf", name="xf")
                nc.sync.dma_start(xf[:], xT_d[k * P : (k + 1) * P, :])
                t = const.tile([P, T], mm_dt, tag=f"xt{k}", name=f"xt{k}")
                nc.vector.tensor_copy(t[:], xf[:])
                xt_sb.append(t)

            # persistent hT and output accumulator
            ht_sb = [
                const.tile([P, T], mm_dt, tag=f"ht{m}", name=f"ht{m}")
                for m in range(H // P)
            ]
            outacc = [
                const.tile([P, T], F32, tag=f"oa{dt}", name=f"oa{dt}")
                for dt in range(D // P)
            ]

            # ---------------- main loop: 4 pairs of 512-row h-groups ----------------
            # L0 per h-group; L1 accumulates a full h-group PAIR (K=1024,
            # 8-matmul chains) per PSUM tile before the SBUF add — fewer
            # chain boundaries and half the evict-adds.
            for pr in range(4):
                w1m_tiles = []
                for sub in range(2):
                    hg = 2 * pr + sub
                    # ---- L0: produce hT[hg*4 .. hg*4+3] ----
                    w0m_tiles = []
                    for pk in range(4):  # d-chunk pairs (k = 2*pk, 2*pk+1)
                        w0f = w0f_pool.tile([P, 1024], F32, tag="w0f", name="w0f")
                        nc.sync.dma_start(
                            w0f[:].rearrange("p (k h) -> p k h", k=2),
                            w0T_d[
                                2 * pk * P : (2 * pk + 2) * P,
                                hg * 512 : (hg + 1) * 512,
                            ].rearrange("(k p) h -> p k h", k=2),
                        )
                        # mask+cast: per 256-col-segment scalar (same d-block
                        # for both chunks of the pair)
                        w0m = w0_pool.tile([P, 1024], mm_dt, tag="w0m", name="w0m")
                        cbase = _DBLK[2 * pk] * 16 + hg * 2
                        nc.vector.tensor_tensor(
                            w0m[:].rearrange("p (k s c) -> p k s c", k=2, c=SEG),
                            w0f[:].rearrange("p (k s c) -> p k s c", k=2, c=SEG),
                            gb[:, cbase : cbase + 2]
                            .unsqueeze(1)
                            .unsqueeze(3)
                            .to_broadcast((P, 2, 2, SEG)),
                            ALU.mult,
                        )
                        w0m_tiles.append(w0m)
                    for ht in range(4):  # h-tiles of 128 within the group
                        m = hg * 4 + ht
                        ps = ps0_pool.tile([P, T], F32, tag="ps0", name="ps0")
                        for k in range(D // P):
                            w0m = w0m_tiles[k // 2]
                            off = (k % 2) * 512 + ht * P
                            nc.tensor.matmul(
                                ps[:],
                                mm_ap(w0m[:, off : off + P]),
                                mm_ap(xt_sb[k][:]),
                                start=(k == 0),
                                stop=(k == D // P - 1),
                            )
                        nc.scalar.activation(
                            ht_sb[m][:], ps[:], AF.Relu, bias=bb0[:, m : m + 1]
                        )
                    # prefetch+mask two w1 pair-tiles per sub-group so the
                    # DMA queue alternates w0/w1 and L1 never waits
                    for pj in (2 * sub, 2 * sub + 1):
                        hc = pr * 8 + 2 * pj
                        w1f = w1f_pool.tile([P, 2048], F32, tag="w1f", name="w1f")
                        nc.sync.dma_start(
                            w1f[:].rearrange("p (k d) -> p k d", k=2),
                            w1T_d[hc * P : (hc + 2) * P, :].rearrange(
                                "(k p) d -> p k d", k=2
                            ),
                        )
                        w1m = w1_pool.tile([P, 2048], mm_dt, tag="w1m", name="w1m")
                        seg_h = hc // 2
                        ap3f = w1f[:].rearrange("p (k d) -> p k d", k=2)
                        ap3m = w1m[:].rearrange("p (k d) -> p k d", k=2)
                        for jd, (c0, c1) in enumerate(
                            [(0, 512), (512, 768), (768, 1024)]
                        ):
                            nc.scalar.activation(
                                ap3m[:, :, c0:c1], ap3f[:, :, c0:c1], AF.Copy,
                                scale=gb[:, jd * 16 + seg_h : jd * 16 + seg_h + 1],
                            )
                        w1m_tiles.append(w1m)

                # ---- L1 partial for this h-group pair (K = 8 x 128) ----
                for dt in range(D // P):  # 8 output d-tiles
                    ps = ps1_pool.tile([P, T], F32, tag="ps1", name="ps1")
                    for j in range(8):  # h-chunks across both groups
                        w1m = w1m_tiles[j // 2]
                        off = (j % 2) * 1024 + dt * P
                        nc.tensor.matmul(
                            ps[:],
                            mm_ap(w1m[:, off : off + P]),
                            mm_ap(ht_sb[pr * 8 + j][:]),
                            start=(j == 0),
                            stop=(j == 7),
                        )
                    if pr == 0:
                        nc.scalar.activation(
                            outacc[dt][:], ps[:], AF.Identity, bias=bb1[:, dt : dt + 1]
                        )
                    else:
                        nc.vector.tensor_tensor(
                            outacc[dt][:], ps[:], outacc[dt][:], ALU.add
                        )

            # ---------------- store ----------------
            for dt in range(D // P):
                nc.sync.dma_start(out_d[dt * P : (dt + 1) * P, :], outacc[dt][:])

    nc.compile()
    return nc


_NC_CACHE: dict[str, bass.Bass] = {}


def _get_nc() -> bass.Bass:
    if MM_DTYPE not in _NC_CACHE:
        _NC_CACHE[MM_DTYPE] = _build_nc()
    return _NC_CACHE[MM_DTYPE]


def make_in_maps(x, w0, b0, w1, b1, arch_embed, arch_mlp):
    """Host-side layout prep (pure reshape/transpose/tile, no arithmetic)."""
    w0T = np.ascontiguousarray(w0.T)                       # [D, H]
    w1T = np.ascontiguousarray(w1.T)                       # [H, D]
    b0r = np.ascontiguousarray(b0.reshape(H // P, P).T)    # [P, 32]
    b1r = np.ascontiguousarray(b1.reshape(D // P, P).T)    # [P, 8]
    ae9 = np.ascontiguousarray(np.repeat(arch_embed, 3).reshape(9, 1))
    am9 = np.ascontiguousarray(np.tile(arch_mlp, 3).reshape(9, 1))
    x3 = x.reshape(N_CORES, T, D)
    return [
        {
            "xT": np.ascontiguousarray(x3[c].T),           # [D, T]
            "w0T": w0T,
            "w1T": w1T,
            "b0r": b0r,
            "b1r": b1r,
            "ae9": ae9,
            "am9": am9,
            "k2": _K2,
        }
        for c in range(N_CORES)
    ]


def kernel(x, w0, b0, w1, b1, arch_embed, arch_mlp):
    x = np.asarray(x, dtype=np.float32)
    w0 = np.asarray(w0, dtype=np.float32)
    b0 = np.asarray(b0, dtype=np.float32)
    w1 = np.asarray(w1, dtype=np.float32)
    b1 = np.asarray(b1, dtype=np.float32)
    arch_embed = np.asarray(arch_embed, dtype=np.float32)
    arch_mlp = np.asarray(arch_mlp, dtype=np.float32)

    in_maps = make_in_maps(x, w0, b0, w1, b1, arch_embed, arch_mlp)
    nc = _get_nc()
    res = run_bass_kernel_spmd(nc, in_maps, core_ids=list(range(N_CORES)))
    out = np.stack([res.results[c]["outT"].T for c in range(N_CORES)], axis=0)
    return np.ascontiguousarray(out)  # [8, 512, 1024] float32


# revision 17
# speedup vs baseline: 1.0022x; 1.0022x over previous
"""Trainium2 Bass kernel for nn_MixedFeedFoward (DARTS-style mixed-architecture MLP).

Math: out = relu(x @ (m0*w0).T + bm0*b0) @ (m1*w1).T + bm1*b1
The DARTS masks are rank-structured.  With a = softmax(arch_embed),
b = softmax(arch_mlp), EMBED = (512,768,1024), RATIO = (2,3,4):

  s_e[h]     = sum_r b_r * [h < e*r]
  g_j[h]     = sum_{e_idx >= j} a_e * s_e[h]
  c_j        = sum_{e_idx >= j} a_e
  W0eff[h,d] = w0[h,d] * g_{blk(d)}[h]      blk(d): 0 for d<512, 1 for d<768, else 2
  bm0[h]     = g_0[h]
  W1eff[d,h] = w1[d,h] * g_{blk(d)}[h]
  bm1[d]     = c_{blk(d)}

g_j is constant on 256-aligned h segments, so masking reduces to 51 runtime
scalars (3 j's x 16 segments + 3 c's), computed on device from the arch inputs
via one tiny matmul against a constant 0/1 selection table.

Sharding: data-parallel over the 4096 tokens -> 512 tokens per core; every core
streams the full weights (cast fp32->bf16 in the DMA engines).  Layer 0
computes hT [H, T] (weights stationary, tokens moving); layer 1 consumes hT
per 512-row h-group, accumulating outT [D, T] partials in PSUM per group and
summing groups into an SBUF fp32 accumulator.
"""

import os

import numpy as np

import concourse.bass as bass
import concourse.mybir as mybir
from concourse import bacc
from concourse.bass_utils import run_bass_kernel_spmd
from concourse.tile import TileContext

N_CORES = 8
D = 1024          # embed dim
H = 4096          # expansion dim
T = 512           # tokens per core (4096 total / 8 cores)
P = 128
SEG = 256         # h-segment size on which g_j is constant
NSEG = H // SEG   # 16
EMBED = (512, 768, 1024)
RATIO = (2, 3, 4)

F32 = mybir.dt.float32
BF16 = mybir.dt.bfloat16
AF = mybir.ActivationFunctionType
ALU = mybir.AluOpType

MM_DTYPE = os.environ.get("BASS_MM_DTYPE", "bf16")


def _build_k2() -> np.ndarray:
    """Constant 0/1 selection table: G_flat[col] = sum_i P9[i] * K2[i, col]
    where P9[e*3+r] = a_e * b_r.
    cols 0..47: col = j*16 + seg -> [e_idx >= j] * [seg*SEG < e*r]
    cols 48..50: col = 48 + j   -> [e_idx >= j]   (since sum_r b_r = 1)
    """
    k2 = np.zeros((9, 51), dtype=np.float32)
    for ie, e in enumerate(EMBED):
        for ir, r in enumerate(RATIO):
            i = ie * 3 + ir
            for j in range(3):
                if ie >= j:
                    for seg in range(NSEG):
                        if seg * SEG < e * r:
                            k2[i, j * 16 + seg] = 1.0
                    k2[i, 48 + j] = 1.0
    return k2


_K2 = _build_k2()

# d-block of each 128-wide d-chunk (0..7): [0,512)->0, [512,768)->1, [768,1024)->2
_DBLK = [0, 0, 0, 0, 1, 1, 2, 2]


def _build_nc() -> bass.Bass:
    nc = bacc.Bacc("TRN2", target_bir_lowering=False, debug=False)

    xT_d = nc.dram_tensor("xT", [D, T], F32, kind="ExternalInput")
    w0T_d = nc.dram_tensor("w0T", [D, H], F32, kind="ExternalInput")
    w1T_d = nc.dram_tensor("w1T", [H, D], F32, kind="ExternalInput")
    b0r_d = nc.dram_tensor("b0r", [P, H // P], F32, kind="ExternalInput")
    b1r_d = nc.dram_tensor("b1r", [P, D // P], F32, kind="ExternalInput")
    ae9_d = nc.dram_tensor("ae9", [9, 1], F32, kind="ExternalInput")
    am9_d = nc.dram_tensor("am9", [9, 1], F32, kind="ExternalInput")
    k2_d = nc.dram_tensor("k2", [9, 51], F32, kind="ExternalInput")
    out_d = nc.dram_tensor("outT", [D, T], F32, kind="ExternalOutput")
    gsc_d = nc.dram_tensor("gscratch", [51], F32)

    mm_dt = BF16 if MM_DTYPE == "bf16" else F32

    def mm_ap(ap):
        return ap.bitcast(mybir.dt.float32r) if MM_DTYPE == "f32r" else ap

    xdma = nc.gpsimd if mm_dt == BF16 else nc.sync  # cast-DMA needs SWDGE

    with TileContext(nc) as tc:
        with (
            tc.tile_pool(name="const", bufs=1) as const,
            tc.tile_pool(name="w0f", bufs=8) as w0f_pool,
            tc.tile_pool(name="xfp", bufs=3) as xf_pool,
            tc.tile_pool(name="w0p", bufs=14) as w0_pool,
            tc.tile_pool(name="w1f", bufs=6) as w1f_pool,
            tc.tile_pool(name="w1p", bufs=6) as w1_pool,
            tc.tile_pool(name="ps0", bufs=3, space="PSUM") as ps0_pool,
            tc.tile_pool(name="ps1", bufs=3, space="PSUM") as ps1_pool,
            tc.tile_pool(name="psp", bufs=1, space="PSUM") as psp_pool,
        ):
            # ---------------- arch-weight prep (tiny) ----------------
            # E9[e*3+r] = exp(ae[e] + am[r]);  sum(E9) = sum_e exp(ae)*sum_r exp(am)
            ae9 = const.tile([9, 1], F32, tag="ae9")
            am9 = const.tile([9, 1], F32, tag="am9")
            nc.gpsimd.dma_start(ae9[:], ae9_d[:, :])
            nc.gpsimd.dma_start(am9[:], am9_d[:, :])
            v9 = const.tile([9, 1], F32, tag="v9")
            nc.vector.tensor_tensor(v9[:], ae9[:], am9[:], ALU.add)
            e9 = const.tile([9, 1], F32, tag="e9")
            nc.scalar.activation(e9[:], v9[:], AF.Exp)
            ones9 = const.tile([9, 1], F32, tag="ones9")
            nc.vector.memset(ones9[:], 1.0)
            s_ps = psp_pool.tile([128, 51], F32, tag="psp", name="s_ps")[0:1, 0:1]
            nc.tensor.matmul(s_ps[:], ones9[:], e9[:], start=True, stop=True)
            s_sb = const.tile([1, 1], F32, tag="s_sb")
            nc.vector.tensor_copy(s_sb[:], s_ps[:])
            rec = const.tile([1, 1], F32, tag="rec")
            nc.vector.reciprocal(rec[:], s_sb[:])

            k2_sb = const.tile([9, 51], F32, tag="k2_sb")
            nc.gpsimd.dma_start(k2_sb[:], k2_d[:, :])
            g_ps = psp_pool.tile([128, 51], F32, tag="psp", name="g_ps")[0:1, :]
            nc.tensor.matmul(g_ps[:], e9[:], k2_sb[:], start=True, stop=True)
            g_row = const.tile([1, 51], F32, tag="g_row")
            nc.vector.tensor_scalar(g_row[:], g_ps[:], rec[0:1, 0:1], None, ALU.mult)
            # broadcast to all 128 partitions via a k=1 ones-matmul
            ones128 = const.tile([1, P], F32, tag="ones128")
            nc.vector.memset(ones128[:], 1.0)
            gb_ps = psp_pool.tile([128, 51], F32, tag="psp", name="gb_ps")
            nc.tensor.matmul(gb_ps[:], ones128[:], g_row[:], start=True, stop=True)
            gb = const.tile([P, 51], F32, tag="gb")
            nc.vector.tensor_copy(gb[:], gb_ps[:])

            # ---------------- effective biases ----------------
            b0_sb = const.tile([P, H // P], F32, tag="b0_sb")
            nc.gpsimd.dma_start(b0_sb[:], b0r_d[:, :])
            bb0 = const.tile([P, H // P], F32, tag="bb0")
            nc.vector.tensor_tensor(
                bb0[:].rearrange("p (s i) -> p s i", i=2),
                b0_sb[:].rearrange("p (s i) -> p s i", i=2),
                gb[:, 0:16].unsqueeze(2).to_broadcast((P, 16, 2)),
                ALU.mult,
            )
            b1_sb = const.tile([P, D // P], F32, tag="b1_sb")
            nc.gpsimd.dma_start(b1_sb[:], b1r_d[:, :])
            bb1 = const.tile([P, D // P], F32, tag="bb1")
            for j, (c0, c1) in enumerate([(0, 4), (4, 6), (6, 8)]):
                nc.vector.tensor_scalar(
                    bb1[:, c0:c1], b1_sb[:, c0:c1],
                    gb[:, 48 + j : 49 + j], None, ALU.mult,
                )

            # ---------------- PE warmup ----------------
            # Keep the PE busy from ~t+4us so the HAM clock gate reaches
            # K=8/8 before the first real matmuls and stays warm through
            # the initial weight-DMA ramp (otherwise the first ~20us of
            # matmuls run at 1.2 GHz).
            junk_w = const.tile([P, 2 * P], mm_dt, tag="junk_w")
            nc.vector.memset(junk_w[:], 0.0)
            junk_x = const.tile([P, T], mm_dt, tag="junk_x")
            nc.vector.memset(junk_x[:], 0.0)
            ps_w = psp_pool.tile([P, T], F32, tag="warm", name="ps_w")
            NWARM = 20
            for i in range(NWARM):
                # alternate weight slices so LDWEIGHTS prefetches into the
                # background buffer and the PE sees full-duty matmul activity
                sl = (i % 2) * P
                nc.tensor.matmul(
                    ps_w[:], mm_ap(junk_w[:, sl : sl + P]), mm_ap(junk_x[:]),
                    start=(i == 0), stop=(i == NWARM - 1),
                )

            # ---------------- xT load (HWDGE fp32 + DVE cast) ----------------
            # x gates every L0 chain; keep it on the fast HWDGE queue, first.
            xt_sb = []
            for k in range(D // P):
                xf = xf_pool.tile([P, T], F32, tag=f"xf{k}", name=f"xf{k}", bufs=1)
                nc.sync.dma_start(xf[:], xT_d[k * P : (k + 1) * P, :])
                t = const.tile([P, T], mm_dt, tag=f"xt{k}", name=f"xt{k}")
                nc.vector.tensor_copy(t[:], xf[:])
                xt_sb.append(t)

            # persistent hT and output accumulator
            ht_sb = [
                const.tile([P, T], mm_dt, tag=f"ht{m}", name=f"ht{m}")
                for m in range(H // P)
            ]
            outacc = [
                const.tile([P, T], F32, tag=f"oa{dt}", name=f"oa{dt}")
                for dt in range(D // P)
            ]

            # ---------------- main loop: 4 pairs of 512-row h-groups ----------------
            # L0 per h-group; L1 accumulates a full h-group PAIR (K=1024,
            # 8-matmul chains) per PSUM tile before the SBUF add — fewer
            # chain boundaries and half the evict-adds.
            for pr in range(4):
                w1m_tiles = []
                for sub in range(2):
                    hg = 2 * pr + sub
                    # ---- L0: produce hT[hg*4 .. hg*4+3] ----
                    w0m_tiles = []
                    for pk in range(4):  # d-chunk pairs (k = 2*pk, 2*pk+1)
                        w0f = w0f_pool.tile([P, 1024], F32, tag="w0f", name="w0f")
                        nc.sync.dma_start(
                            w0f[:].rearrange("p (k h) -> p k h", k=2),
                            w0T_d[
                                2 * pk * P : (2 * pk + 2) * P,
                                hg * 512 : (hg + 1) * 512,
                            ].rearrange("(k p) h -> p k h", k=2),
                        )
                        # mask+cast: per 256-col-segment scalar (same d-block
                        # for both chunks of the pair)
                        w0m = w0_pool.tile([P, 1024], mm_dt, tag="w0m", name="w0m")
                        cbase = _DBLK[2 * pk] * 16 + hg * 2
                        nc.vector.tensor_tensor(
                            w0m[:].rearrange("p (k s c) -> p k s c", k=2, c=SEG),
                            w0f[:].rearrange("p (k s c) -> p k s c", k=2, c=SEG),
                            gb[:, cbase : cbase + 2]
                            .unsqueeze(1)
                            .unsqueeze(3)
                            .to_broadcast((P, 2, 2, SEG)),
                            ALU.mult,
                        )
                        w0m_tiles.append(w0m)
                    for ht in range(4):  # h-tiles of 128 within the group
                        m = hg * 4 + ht
                        ps = ps0_pool.tile([P, T], F32, tag="ps0", name="ps0")
                        for k in range(D // P):
                            w0m = w0m_tiles[k // 2]
                            off = (k % 2) * 512 + ht * P
                            nc.tensor.matmul(
                                ps[:],
                                mm_ap(w0m[:, off : off + P]),
                                mm_ap(xt_sb[k][:]),
                                start=(k == 0),
                                stop=(k == D // P - 1),
                            )
                        nc.scalar.activation(
                            ht_sb[m][:], ps[:], AF.Relu, bias=bb0[:, m : m + 1]
                        )
                    # prefetch+mask two w1 pair-tiles per sub-group so the
                    # DMA queue alternates w0/w1 and L1 never waits
                    for pj in (2 * sub, 2 * sub + 1):
                        hc = pr * 8 + 2 * pj
                        w1f = w1f_pool.tile([P, 2048], F32, tag="w1f", name="w1f")
                        nc.sync.dma_start(
                            w1f[:].rearrange("p (k d) -> p k d", k=2),
                            w1T_d[hc * P : (hc + 2) * P, :].rearrange(
                                "(k p) d -> p k d", k=2
                            ),
                        )
                        w1m = w1_pool.tile([P, 2048], mm_dt, tag="w1m", name="w1m")
                        seg_h = hc // 2
                        ap3f = w1f[:].rearrange("p (k d) -> p k d", k=2)
                        ap3m = w1m[:].rearrange("p (k d) -> p k d", k=2)
                        for jd, (c0, c1) in enumerate(
                            [(0, 512), (512, 768), (768, 1024)]
                        ):
                            nc.scalar.activation(
                                ap3m[:, :, c0:c1], ap3f[:, :, c0:c1], AF.Copy,
                                scale=gb[:, jd * 16 + seg_h : jd * 16 + seg_h + 1],
                            )
                        w1m_tiles.append(w1m)

                # ---- L1 partial for this h-group pair (K = 8 x 128) ----
                for dt in range(D // P):  # 8 output d-tiles
                    ps = ps1_pool.tile([P, T], F32, tag="ps1", name="ps1")
                    for j in range(8):  # h-chunks across both groups
                        w1m = w1m_tiles[j // 2]
                        off = (j % 2) * 1024 + dt * P
                        nc.tensor.matmul(
                            ps[:],
                            mm_ap(w1m[:, off : off + P]),
                            mm_ap(ht_sb[pr * 8 + j][:]),
                            start=(j == 0),
                            stop=(j == 7),
                        )
                    if pr == 0:
                        nc.scalar.activation(
                            outacc[dt][:], ps[:], AF.Identity, bias=bb1[:, dt : dt + 1]
                        )
                    else:
                        nc.vector.tensor_tensor(
                            outacc[dt][:], ps[:], outacc[dt][:], ALU.add
                        )

            # ---------------- store ----------------
            for dt in range(D // P):
                nc.sync.dma_start(out_d[dt * P : (dt + 1) * P, :], outacc[dt][:])

    nc.compile()
    return nc


_NC_CACHE: dict[str, bass.Bass] = {}


def _get_nc() -> bass.Bass:
    if MM_DTYPE not in _NC_CACHE:
        _NC_CACHE[MM_DTYPE] = _build_nc()
    return _NC_CACHE[MM_DTYPE]


def make_in_maps(x, w0, b0, w1, b1, arch_embed, arch_mlp):
    """Host-side layout prep (pure reshape/transpose/tile, no arithmetic)."""
    w0T = np.ascontiguousarray(w0.T)                       # [D, H]
    w1T = np.ascontiguousarray(w1.T)                       # [H, D]
    b0r = np.ascontiguousarray(b0.reshape(H // P, P).T)    # [P, 32]
    b1r = np.ascontiguousarray(b1.reshape(D // P, P).T)    # [P, 8]
    ae9 = np.ascontiguousarray(np.repeat(arch_embed, 3).reshape(9, 1))
    am9 = np.ascontiguousarray(np.tile(arch_mlp, 3).reshape(9, 1))
    x3 = x.reshape(N_CORES, T, D)
    return [
        {
            "xT": np.ascontiguousarray(x3[c].T),           # [D, T]
            "w0T": w0T,
            "w1T": w1T,
            "b0r": b0r,
            "b1r": b1r,
            "ae9": ae9,
            "am9": am9,
            "k2": _K2,
        }
        for c in range(N_CORES)
    ]


def kernel(x, w0, b0, w1, b1, arch_embed, arch_mlp):
    x = np.asarray(x, dtype=np.float32)
    w0 = np.asarray(w0, dtype=np.float32)
    b0 = np.asarray(b0, dtype=np.float32)
    w1 = np.asarray(w1, dtype=np.float32)
    b1 = np.asarray(b1, dtype=np.float32)
    arch_embed = np.asarray(arch_embed, dtype=np.float32)
    arch_mlp = np.asarray(arch_mlp, dtype=np.float32)

    in_maps = make_in_maps(x, w0, b0, w1, b1, arch_embed, arch_mlp)
    nc = _get_nc()
    res = run_bass_kernel_spmd(nc, in_maps, core_ids=list(range(N_CORES)))
    out = np.stack([res.results[c]["outT"].T for c in range(N_CORES)], axis=0)
    return np.ascontiguousarray(out)  # [8, 512, 1024] float32


# revision 18
# speedup vs baseline: 1.0079x; 1.0057x over previous
"""Trainium2 Bass kernel for nn_MixedFeedFoward (DARTS-style mixed-architecture MLP).

Math: out = relu(x @ (m0*w0).T + bm0*b0) @ (m1*w1).T + bm1*b1
The DARTS masks are rank-structured.  With a = softmax(arch_embed),
b = softmax(arch_mlp), EMBED = (512,768,1024), RATIO = (2,3,4):

  s_e[h]     = sum_r b_r * [h < e*r]
  g_j[h]     = sum_{e_idx >= j} a_e * s_e[h]
  c_j        = sum_{e_idx >= j} a_e
  W0eff[h,d] = w0[h,d] * g_{blk(d)}[h]      blk(d): 0 for d<512, 1 for d<768, else 2
  bm0[h]     = g_0[h]
  W1eff[d,h] = w1[d,h] * g_{blk(d)}[h]
  bm1[d]     = c_{blk(d)}

g_j is constant on 256-aligned h segments, so masking reduces to 51 runtime
scalars (3 j's x 16 segments + 3 c's), computed on device from the arch inputs
via one tiny matmul against a constant 0/1 selection table.

Sharding: data-parallel over the 4096 tokens -> 512 tokens per core; every core
streams the full weights (cast fp32->bf16 in the DMA engines).  Layer 0
computes hT [H, T] (weights stationary, tokens moving); layer 1 consumes hT
per 512-row h-group, accumulating outT [D, T] partials in PSUM per group and
summing groups into an SBUF fp32 accumulator.
"""

import os

import numpy as np

import concourse.bass as bass
import concourse.mybir as mybir
from concourse import bacc
from concourse.bass_utils import run_bass_kernel_spmd
from concourse.tile import TileContext

N_CORES = 8
D = 1024          # embed dim
H = 4096          # expansion dim
T = 512           # tokens per core (4096 total / 8 cores)
P = 128
SEG = 256         # h-segment size on which g_j is constant
NSEG = H // SEG   # 16
EMBED = (512, 768, 1024)
RATIO = (2, 3, 4)

F32 = mybir.dt.float32
BF16 = mybir.dt.bfloat16
AF = mybir.ActivationFunctionType
ALU = mybir.AluOpType

MM_DTYPE = os.environ.get("BASS_MM_DTYPE", "bf16")


def _build_k2() -> np.ndarray:
    """Constant 0/1 selection table: G_flat[col] = sum_i P9[i] * K2[i, col]
    where P9[e*3+r] = a_e * b_r.
    cols 0..47: col = j*16 + seg -> [e_idx >= j] * [seg*SEG < e*r]
    cols 48..50: col = 48 + j   -> [e_idx >= j]   (since sum_r b_r = 1)
    """
    k2 = np.zeros((9, 51), dtype=np.float32)
    for ie, e in enumerate(EMBED):
        for ir, r in enumerate(RATIO):
            i = ie * 3 + ir
            for j in range(3):
                if ie >= j:
                    for seg in range(NSEG):
                        if seg * SEG < e * r:
                            k2[i, j * 16 + seg] = 1.0
                    k2[i, 48 + j] = 1.0
    return k2


_K2 = _build_k2()

# d-block of each 128-wide d-chunk (0..7): [0,512)->0, [512,768)->1, [768,1024)->2
_DBLK = [0, 0, 0, 0, 1, 1, 2, 2]


def _build_nc() -> bass.Bass:
    nc = bacc.Bacc("TRN2", target_bir_lowering=False, debug=False)

    xT_d = nc.dram_tensor("xT", [D, T], F32, kind="ExternalInput")
    w0T_d = nc.dram_tensor("w0T", [D, H], F32, kind="ExternalInput")
    w1T_d = nc.dram_tensor("w1T", [H, D], F32, kind="ExternalInput")
    b0r_d = nc.dram_tensor("b0r", [P, H // P], F32, kind="ExternalInput")
    b1r_d = nc.dram_tensor("b1r", [P, D // P], F32, kind="ExternalInput")
    ae9_d = nc.dram_tensor("ae9", [9, 1], F32, kind="ExternalInput")
    am9_d = nc.dram_tensor("am9", [9, 1], F32, kind="ExternalInput")
    k2_d = nc.dram_tensor("k2", [9, 51], F32, kind="ExternalInput")
    out_d = nc.dram_tensor("outT", [D, T], F32, kind="ExternalOutput")
    gsc_d = nc.dram_tensor("gscratch", [51], F32)

    mm_dt = BF16 if MM_DTYPE == "bf16" else F32

    def mm_ap(ap):
        return ap.bitcast(mybir.dt.float32r) if MM_DTYPE == "f32r" else ap

    xdma = nc.gpsimd if mm_dt == BF16 else nc.sync  # cast-DMA needs SWDGE

    with TileContext(nc) as tc:
        with (
            tc.tile_pool(name="const", bufs=1) as const,
            tc.tile_pool(name="w0f", bufs=8) as w0f_pool,
            tc.tile_pool(name="xfp", bufs=3) as xf_pool,
            tc.tile_pool(name="w0p", bufs=12) as w0_pool,
            tc.tile_pool(name="w1f", bufs=6) as w1f_pool,
            tc.tile_pool(name="w1p", bufs=6) as w1_pool,
            tc.tile_pool(name="ps0", bufs=3, space="PSUM") as ps0_pool,
            tc.tile_pool(name="ps1", bufs=3, space="PSUM") as ps1_pool,
            tc.tile_pool(name="psp", bufs=1, space="PSUM") as psp_pool,
        ):
            # ---------------- arch-weight prep (tiny) ----------------
            # E9[e*3+r] = exp(ae[e] + am[r]);  sum(E9) = sum_e exp(ae)*sum_r exp(am)
            ae9 = const.tile([9, 1], F32, tag="ae9")
            am9 = const.tile([9, 1], F32, tag="am9")
            nc.gpsimd.dma_start(ae9[:], ae9_d[:, :])
            nc.gpsimd.dma_start(am9[:], am9_d[:, :])
            v9 = const.tile([9, 1], F32, tag="v9")
            nc.vector.tensor_tensor(v9[:], ae9[:], am9[:], ALU.add)
            e9 = const.tile([9, 1], F32, tag="e9")
            nc.scalar.activation(e9[:], v9[:], AF.Exp)
            ones9 = const.tile([9, 1], F32, tag="ones9")
            nc.vector.memset(ones9[:], 1.0)
            s_ps = psp_pool.tile([128, 51], F32, tag="psp", name="s_ps")[0:1, 0:1]
            nc.tensor.matmul(s_ps[:], ones9[:], e9[:], start=True, stop=True)
            s_sb = const.tile([1, 1], F32, tag="s_sb")
            nc.vector.tensor_copy(s_sb[:], s_ps[:])
            rec = const.tile([1, 1], F32, tag="rec")
            nc.vector.reciprocal(rec[:], s_sb[:])

            k2_sb = const.tile([9, 51], F32, tag="k2_sb")
            nc.gpsimd.dma_start(k2_sb[:], k2_d[:, :])
            g_ps = psp_pool.tile([128, 51], F32, tag="psp", name="g_ps")[0:1, :]
            nc.tensor.matmul(g_ps[:], e9[:], k2_sb[:], start=True, stop=True)
            g_row = const.tile([1, 51], F32, tag="g_row")
            nc.vector.tensor_scalar(g_row[:], g_ps[:], rec[0:1, 0:1], None, ALU.mult)
            # broadcast to all 128 partitions via a k=1 ones-matmul
            ones128 = const.tile([1, P], F32, tag="ones128")
            nc.vector.memset(ones128[:], 1.0)
            gb_ps = psp_pool.tile([128, 51], F32, tag="psp", name="gb_ps")
            nc.tensor.matmul(gb_ps[:], ones128[:], g_row[:], start=True, stop=True)
            gb = const.tile([P, 51], F32, tag="gb")
            nc.vector.tensor_copy(gb[:], gb_ps[:])

            # ---------------- effective biases ----------------
            b0_sb = const.tile([P, H // P], F32, tag="b0_sb")
            nc.gpsimd.dma_start(b0_sb[:], b0r_d[:, :])
            bb0 = const.tile([P, H // P], F32, tag="bb0")
            nc.vector.tensor_tensor(
                bb0[:].rearrange("p (s i) -> p s i", i=2),
                b0_sb[:].rearrange("p (s i) -> p s i", i=2),
                gb[:, 0:16].unsqueeze(2).to_broadcast((P, 16, 2)),
                ALU.mult,
            )
            b1_sb = const.tile([P, D // P], F32, tag="b1_sb")
            nc.gpsimd.dma_start(b1_sb[:], b1r_d[:, :])
            bb1 = const.tile([P, D // P], F32, tag="bb1")
            for j, (c0, c1) in enumerate([(0, 4), (4, 6), (6, 8)]):
                nc.vector.tensor_scalar(
                    bb1[:, c0:c1], b1_sb[:, c0:c1],
                    gb[:, 48 + j : 49 + j], None, ALU.mult,
                )

            # ---------------- PE warmup ----------------
            # Keep the PE busy from ~t+4us so the HAM clock gate reaches
            # K=8/8 before the first real matmuls and stays warm through
            # the initial weight-DMA ramp (otherwise the first ~20us of
            # matmuls run at 1.2 GHz).
            junk_w = const.tile([P, 2 * P], mm_dt, tag="junk_w")
            nc.vector.memset(junk_w[:], 0.0)
            junk_x = const.tile([P, T], mm_dt, tag="junk_x")
            nc.vector.memset(junk_x[:], 0.0)
            ps_w = psp_pool.tile([P, T], F32, tag="warm", name="ps_w")
            NWARM = 24
            for i in range(NWARM):
                # alternate weight slices so LDWEIGHTS prefetches into the
                # background buffer and the PE sees full-duty matmul activity
                sl = (i % 2) * P
                nc.tensor.matmul(
                    ps_w[:], mm_ap(junk_w[:, sl : sl + P]), mm_ap(junk_x[:]),
                    start=(i == 0), stop=(i == NWARM - 1),
                )

            # ---------------- xT load (HWDGE fp32 + DVE cast) ----------------
            # x gates every L0 chain; keep it on the fast HWDGE queue, first.
            xt_sb = []
            for k in range(D // P):
                xf = xf_pool.tile([P, T], F32, tag=f"xf{k}", name=f"xf{k}", bufs=1)
                nc.sync.dma_start(xf[:], xT_d[k * P : (k + 1) * P, :])
                t = const.tile([P, T], mm_dt, tag=f"xt{k}", name=f"xt{k}")
                nc.vector.tensor_copy(t[:], xf[:])
                xt_sb.append(t)

            # persistent hT and output accumulator
            ht_sb = [
                const.tile([P, T], mm_dt, tag=f"ht{m}", name=f"ht{m}")
                for m in range(H // P)
            ]
            outacc = [
                const.tile([P, T], F32, tag=f"oa{dt}", name=f"oa{dt}")
                for dt in range(D // P)
            ]

            # ---------------- main loop: 4 pairs of 512-row h-groups ----------------
            # L0 per h-group; L1 accumulates a full h-group PAIR (K=1024,
            # 8-matmul chains) per PSUM tile before the SBUF add — fewer
            # chain boundaries and half the evict-adds.
            for pr in range(4):
                w1m_tiles = []
                for sub in range(2):
                    hg = 2 * pr + sub
                    # ---- L0: produce hT[hg*4 .. hg*4+3] ----
                    w0m_tiles = []
                    for pk in range(4):  # d-chunk pairs (k = 2*pk, 2*pk+1)
                        w0f = w0f_pool.tile([P, 1024], F32, tag="w0f", name="w0f")
                        nc.sync.dma_start(
                            w0f[:].rearrange("p (k h) -> p k h", k=2),
                            w0T_d[
                                2 * pk * P : (2 * pk + 2) * P,
                                hg * 512 : (hg + 1) * 512,
                            ].rearrange("(k p) h -> p k h", k=2),
                        )
                        # mask+cast: per 256-col-segment scalar (same d-block
                        # for both chunks of the pair)
                        w0m = w0_pool.tile([P, 1024], mm_dt, tag="w0m", name="w0m")
                        cbase = _DBLK[2 * pk] * 16 + hg * 2
                        nc.vector.tensor_tensor(
                            w0m[:].rearrange("p (k s c) -> p k s c", k=2, c=SEG),
                            w0f[:].rearrange("p (k s c) -> p k s c", k=2, c=SEG),
                            gb[:, cbase : cbase + 2]
                            .unsqueeze(1)
                            .unsqueeze(3)
                            .to_broadcast((P, 2, 2, SEG)),
                            ALU.mult,
                        )
                        w0m_tiles.append(w0m)
                    for ht in range(4):  # h-tiles of 128 within the group
                        m = hg * 4 + ht
                        ps = ps0_pool.tile([P, T], F32, tag="ps0", name="ps0")
                        for k in range(D // P):
                            w0m = w0m_tiles[k // 2]
                            off = (k % 2) * 512 + ht * P
                            nc.tensor.matmul(
                                ps[:],
                                mm_ap(w0m[:, off : off + P]),
                                mm_ap(xt_sb[k][:]),
                                start=(k == 0),
                                stop=(k == D // P - 1),
                            )
                        nc.scalar.activation(
                            ht_sb[m][:], ps[:], AF.Relu, bias=bb0[:, m : m + 1]
                        )
                    # prefetch+mask two w1 pair-tiles per sub-group so the
                    # DMA queue alternates w0/w1 and L1 never waits
                    for pj in (2 * sub, 2 * sub + 1):
                        hc = pr * 8 + 2 * pj
                        w1f = w1f_pool.tile([P, 2048], F32, tag="w1f", name="w1f")
                        nc.sync.dma_start(
                            w1f[:].rearrange("p (k d) -> p k d", k=2),
                            w1T_d[hc * P : (hc + 2) * P, :].rearrange(
                                "(k p) d -> p k d", k=2
                            ),
                        )
                        w1m = w1_pool.tile([P, 2048], mm_dt, tag="w1m", name="w1m")
                        seg_h = hc // 2
                        ap3f = w1f[:].rearrange("p (k d) -> p k d", k=2)
                        ap3m = w1m[:].rearrange("p (k d) -> p k d", k=2)
                        for jd, (c0, c1) in enumerate(
                            [(0, 512), (512, 768), (768, 1024)]
                        ):
                            nc.scalar.activation(
                                ap3m[:, :, c0:c1], ap3f[:, :, c0:c1], AF.Copy,
                                scale=gb[:, jd * 16 + seg_h : jd * 16 + seg_h + 1],
                            )
                        w1m_tiles.append(w1m)

                # ---- L1 partial for this h-group pair (K = 8 x 128) ----
                for dt in range(D // P):  # 8 output d-tiles
                    ps = ps1_pool.tile([P, T], F32, tag="ps1", name="ps1")
                    for j in range(8):  # h-chunks across both groups
                        w1m = w1m_tiles[j // 2]
                        off = (j % 2) * 1024 + dt * P
                        nc.tensor.matmul(
                            ps[:],
                            mm_ap(w1m[:, off : off + P]),
                            mm_ap(ht_sb[pr * 8 + j][:]),
                            start=(j == 0),
                            stop=(j == 7),
                        )
                    if pr == 0:
                        nc.scalar.activation(
                            outacc[dt][:], ps[:], AF.Identity, bias=bb1[:, dt : dt + 1]
                        )
                    else:
                        nc.vector.tensor_tensor(
                            outacc[dt][:], ps[:], outacc[dt][:], ALU.add
                        )

            # ---------------- store ----------------
            for dt in range(D // P):
                nc.sync.dma_start(out_d[dt * P : (dt + 1) * P, :], outacc[dt][:])

    nc.compile()
    return nc


_NC_CACHE: dict[str, bass.Bass] = {}


def _get_nc() -> bass.Bass:
    if MM_DTYPE not in _NC_CACHE:
        _NC_CACHE[MM_DTYPE] = _build_nc()
    return _NC_CACHE[MM_DTYPE]


def make_in_maps(x, w0, b0, w1, b1, arch_embed, arch_mlp):
    """Host-side layout prep (pure reshape/transpose/tile, no arithmetic)."""
    w0T = np.ascontiguousarray(w0.T)                       # [D, H]
    w1T = np.ascontiguousarray(w1.T)                       # [H, D]
    b0r = np.ascontiguousarray(b0.reshape(H // P, P).T)    # [P, 32]
    b1r = np.ascontiguousarray(b1.reshape(D // P, P).T)    # [P, 8]
    ae9 = np.ascontiguousarray(np.repeat(arch_embed, 3).reshape(9, 1))
    am9 = np.ascontiguousarray(np.tile(arch_mlp, 3).reshape(9, 1))
    x3 = x.reshape(N_CORES, T, D)
    return [
        {
            "xT": np.ascontiguousarray(x3[c].T),           # [D, T]
            "w0T": w0T,
            "w1T": w1T,
            "b0r": b0r,
            "b1r": b1r,
            "ae9": ae9,
            "am9": am9,
            "k2": _K2,
        }
        for c in range(N_CORES)
    ]


def kernel(x, w0, b0, w1, b1, arch_embed, arch_mlp):
    x = np.asarray(x, dtype=np.float32)
    w0 = np.asarray(w0, dtype=np.float32)
    b0 = np.asarray(b0, dtype=np.float32)
    w1 = np.asarray(w1, dtype=np.float32)
    b1 = np.asarray(b1, dtype=np.float32)
    arch_embed = np.asarray(arch_embed, dtype=np.float32)
    arch_mlp = np.asarray(arch_mlp, dtype=np.float32)

    in_maps = make_in_maps(x, w0, b0, w1, b1, arch_embed, arch_mlp)
    nc = _get_nc()
    res = run_bass_kernel_spmd(nc, in_maps, core_ids=list(range(N_CORES)))
    out = np.stack([res.results[c]["outT"].T for c in range(N_CORES)], axis=0)
    return np.ascontiguousarray(out)  # [8, 512, 1024] float32


# revision 19
# speedup vs baseline: 1.0143x; 1.0063x over previous
"""Trainium2 Bass kernel for nn_MixedFeedFoward (DARTS-style mixed-architecture MLP).

Math: out = relu(x @ (m0*w0).T + bm0*b0) @ (m1*w1).T + bm1*b1
The DARTS masks are rank-structured.  With a = softmax(arch_embed),
b = softmax(arch_mlp), EMBED = (512,768,1024), RATIO = (2,3,4):

  s_e[h]     = sum_r b_r * [h < e*r]
  g_j[h]     = sum_{e_idx >= j} a_e * s_e[h]
  c_j        = sum_{e_idx >= j} a_e
  W0eff[h,d] = w0[h,d] * g_{blk(d)}[h]      blk(d): 0 for d<512, 1 for d<768, else 2
  bm0[h]     = g_0[h]
  W1eff[d,h] = w1[d,h] * g_{blk(d)}[h]
  bm1[d]     = c_{blk(d)}

g_j is constant on 256-aligned h segments, so masking reduces to 51 runtime
scalars (3 j's x 16 segments + 3 c's), computed on device from the arch inputs
via one tiny matmul against a constant 0/1 selection table.

Sharding: data-parallel over the 4096 tokens -> 512 tokens per core; every core
streams the full weights (cast fp32->bf16 in the DMA engines).  Layer 0
computes hT [H, T] (weights stationary, tokens moving); layer 1 consumes hT
per 512-row h-group, accumulating outT [D, T] partials in PSUM per group and
summing groups into an SBUF fp32 accumulator.
"""

import os

import numpy as np

import concourse.bass as bass
import concourse.mybir as mybir
from concourse import bacc
from concourse.bass_utils import run_bass_kernel_spmd
from concourse.tile import TileContext

N_CORES = 8
D = 1024          # embed dim
H = 4096          # expansion dim
T = 512           # tokens per core (4096 total / 8 cores)
P = 128
SEG = 256         # h-segment size on which g_j is constant
NSEG = H // SEG   # 16
EMBED = (512, 768, 1024)
RATIO = (2, 3, 4)

F32 = mybir.dt.float32
BF16 = mybir.dt.bfloat16
AF = mybir.ActivationFunctionType
ALU = mybir.AluOpType

MM_DTYPE = os.environ.get("BASS_MM_DTYPE", "bf16")


def _build_k2() -> np.ndarray:
    """Constant 0/1 selection table: G_flat[col] = sum_i P9[i] * K2[i, col]
    where P9[e*3+r] = a_e * b_r.
    cols 0..47: col = j*16 + seg -> [e_idx >= j] * [seg*SEG < e*r]
    cols 48..50: col = 48 + j   -> [e_idx >= j]   (since sum_r b_r = 1)
    """
    k2 = np.zeros((9, 51), dtype=np.float32)
    for ie, e in enumerate(EMBED):
        for ir, r in enumerate(RATIO):
            i = ie * 3 + ir
            for j in range(3):
                if ie >= j:
                    for seg in range(NSEG):
                        if seg * SEG < e * r:
                            k2[i, j * 16 + seg] = 1.0
                    k2[i, 48 + j] = 1.0
    return k2


_K2 = _build_k2()

# d-block of each 128-wide d-chunk (0..7): [0,512)->0, [512,768)->1, [768,1024)->2
_DBLK = [0, 0, 0, 0, 1, 1, 2, 2]


def _build_nc() -> bass.Bass:
    nc = bacc.Bacc("TRN2", target_bir_lowering=False, debug=False)

    xT_d = nc.dram_tensor("xT", [D, T], F32, kind="ExternalInput")
    w0T_d = nc.dram_tensor("w0T", [D, H], F32, kind="ExternalInput")
    w1T_d = nc.dram_tensor("w1T", [H, D], F32, kind="ExternalInput")
    b0r_d = nc.dram_tensor("b0r", [P, H // P], F32, kind="ExternalInput")
    b1r_d = nc.dram_tensor("b1r", [P, D // P], F32, kind="ExternalInput")
    ae9_d = nc.dram_tensor("ae9", [9, 1], F32, kind="ExternalInput")
    am9_d = nc.dram_tensor("am9", [9, 1], F32, kind="ExternalInput")
    k2_d = nc.dram_tensor("k2", [9, 51], F32, kind="ExternalInput")
    out_d = nc.dram_tensor("outT", [D, T], F32, kind="ExternalOutput")
    gsc_d = nc.dram_tensor("gscratch", [51], F32)

    F32R = mybir.dt.float32r
    IS_F32R = MM_DTYPE == "f32r"
    mm_dt = F32R if IS_F32R else BF16

    def mm_ap(ap):
        return ap

    def rcast(ap):
        # view a staging f32 AP as f32r (the producing mask op wrote it
        # with an f32r-typed output AP, so the values are rounded)
        return ap.bitcast(F32R)

    with TileContext(nc) as tc:
        with (
            tc.tile_pool(name="const", bufs=1) as const,
            tc.tile_pool(name="w0f", bufs=10 if IS_F32R else 8) as w0f_pool,
            tc.tile_pool(name="xfp", bufs=3) as xf_pool,
            tc.tile_pool(name="w0p", bufs=12) as w0_pool,
            tc.tile_pool(name="w1f", bufs=4 if IS_F32R else 6) as w1f_pool,
            tc.tile_pool(name="w1p", bufs=6) as w1_pool,
            tc.tile_pool(name="ps0", bufs=3, space="PSUM") as ps0_pool,
            tc.tile_pool(name="ps1", bufs=3, space="PSUM") as ps1_pool,
            tc.tile_pool(name="psp", bufs=1, space="PSUM") as psp_pool,
        ):
            # ---------------- arch-weight prep (tiny) ----------------
            # E9[e*3+r] = exp(ae[e] + am[r]);  sum(E9) = sum_e exp(ae)*sum_r exp(am)
            ae9 = const.tile([9, 1], F32, tag="ae9")
            am9 = const.tile([9, 1], F32, tag="am9")
            nc.gpsimd.dma_start(ae9[:], ae9_d[:, :])
            nc.gpsimd.dma_start(am9[:], am9_d[:, :])
            v9 = const.tile([9, 1], F32, tag="v9")
            nc.vector.tensor_tensor(v9[:], ae9[:], am9[:], ALU.add)
            e9 = const.tile([9, 1], F32, tag="e9")
            nc.scalar.activation(e9[:], v9[:], AF.Exp)
            ones9 = const.tile([9, 1], F32, tag="ones9")
            nc.vector.memset(ones9[:], 1.0)
            s_ps = psp_pool.tile([128, 51], F32, tag="psp", name="s_ps")[0:1, 0:1]
            nc.tensor.matmul(s_ps[:], ones9[:], e9[:], start=True, stop=True)
            s_sb = const.tile([1, 1], F32, tag="s_sb")
            nc.vector.tensor_copy(s_sb[:], s_ps[:])
            rec = const.tile([1, 1], F32, tag="rec")
            nc.vector.reciprocal(rec[:], s_sb[:])

            k2_sb = const.tile([9, 51], F32, tag="k2_sb")
            nc.gpsimd.dma_start(k2_sb[:], k2_d[:, :])
            g_ps = psp_pool.tile([128, 51], F32, tag="psp", name="g_ps")[0:1, :]
            nc.tensor.matmul(g_ps[:], e9[:], k2_sb[:], start=True, stop=True)
            g_row = const.tile([1, 51], F32, tag="g_row")
            nc.vector.tensor_scalar(g_row[:], g_ps[:], rec[0:1, 0:1], None, ALU.mult)
            # broadcast to all 128 partitions via a k=1 ones-matmul
            ones128 = const.tile([1, P], F32, tag="ones128")
            nc.vector.memset(ones128[:], 1.0)
            gb_ps = psp_pool.tile([128, 51], F32, tag="psp", name="gb_ps")
            nc.tensor.matmul(gb_ps[:], ones128[:], g_row[:], start=True, stop=True)
            gb = const.tile([P, 51], F32, tag="gb")
            nc.vector.tensor_copy(gb[:], gb_ps[:])

            # ---------------- effective biases ----------------
            b0_sb = const.tile([P, H // P], F32, tag="b0_sb")
            nc.gpsimd.dma_start(b0_sb[:], b0r_d[:, :])
            bb0 = const.tile([P, H // P], F32, tag="bb0")
            nc.vector.tensor_tensor(
                bb0[:].rearrange("p (s i) -> p s i", i=2),
                b0_sb[:].rearrange("p (s i) -> p s i", i=2),
                gb[:, 0:16].unsqueeze(2).to_broadcast((P, 16, 2)),
                ALU.mult,
            )
            b1_sb = const.tile([P, D // P], F32, tag="b1_sb")
            nc.gpsimd.dma_start(b1_sb[:], b1r_d[:, :])
            bb1 = const.tile([P, D // P], F32, tag="bb1")
            for j, (c0, c1) in enumerate([(0, 4), (4, 6), (6, 8)]):
                nc.vector.tensor_scalar(
                    bb1[:, c0:c1], b1_sb[:, c0:c1],
                    gb[:, 48 + j : 49 + j], None, ALU.mult,
                )

            # ---------------- PE warmup ----------------
            # Keep the PE busy from ~t+4us so the HAM clock gate reaches
            # K=8/8 before the first real matmuls and stays warm through
            # the initial weight-DMA ramp (otherwise the first ~20us of
            # matmuls run at 1.2 GHz).
            junk_w = const.tile([P, 2 * P], mm_dt, tag="junk_w")
            nc.vector.memset(junk_w[:], 0.0)
            junk_x = const.tile([P, T], mm_dt, tag="junk_x")
            nc.vector.memset(junk_x[:], 0.0)
            ps_w = psp_pool.tile([P, T], F32, tag="warm", name="ps_w")
            NWARM = 24
            for i in range(NWARM):
                # alternate weight slices so LDWEIGHTS prefetches into the
                # background buffer and the PE sees full-duty matmul activity
                sl = (i % 2) * P
                nc.tensor.matmul(
                    ps_w[:], junk_w[:, sl : sl + P], junk_x[:],
                    start=(i == 0), stop=(i == NWARM - 1),
                )

            # ---------------- xT load (HWDGE fp32 + DVE cast) ----------------
            # x gates every L0 chain; keep it on the fast HWDGE queue, first.
            xt_sb = []
            for k in range(D // P):
                xf = xf_pool.tile([P, T], F32, tag=f"xf{k}", name=f"xf{k}", bufs=1)
                nc.sync.dma_start(xf[:], xT_d[k * P : (k + 1) * P, :])
                t = const.tile([P, T], mm_dt, tag=f"xt{k}", name=f"xt{k}")
                nc.vector.tensor_copy(t[:], xf[:])
                xt_sb.append(t)

            # persistent hT and output accumulator
            ht_sb = [
                const.tile([P, T], mm_dt, tag=f"ht{m}", name=f"ht{m}")
                for m in range(H // P)
            ]
            outacc = [
                const.tile([P, T], F32, tag=f"oa{dt}", name=f"oa{dt}")
                for dt in range(D // P)
            ]

            # ---------------- main loop: 4 pairs of 512-row h-groups ----------------
            # L0 per h-group; L1 accumulates a full h-group PAIR (K=1024,
            # 8-matmul chains) per PSUM tile before the SBUF add — fewer
            # chain boundaries and half the evict-adds.
            for pr in range(4):
                w1m_tiles = []
                for sub in range(2):
                    hg = 2 * pr + sub
                    # ---- L0: produce hT[hg*4 .. hg*4+3] ----
                    w0m_tiles = []
                    for pk in range(4):  # d-chunk pairs (k = 2*pk, 2*pk+1)
                        w0f = w0f_pool.tile([P, 1024], F32, tag="w0f", name="w0f")
                        nc.sync.dma_start(
                            w0f[:].rearrange("p (k h) -> p k h", k=2),
                            w0T_d[
                                2 * pk * P : (2 * pk + 2) * P,
                                hg * 512 : (hg + 1) * 512,
                            ].rearrange("(k p) h -> p k h", k=2),
                        )
                        # mask+cast: per 256-col-segment scalar (same d-block
                        # for both chunks of the pair)
                        cbase = _DBLK[2 * pk] * 16 + hg * 2
                        msk = (
                            gb[:, cbase : cbase + 2]
                            .unsqueeze(1)
                            .unsqueeze(3)
                            .to_broadcast((P, 2, 2, SEG))
                        )
                        ap_in = w0f[:].rearrange("p (k s c) -> p k s c", k=2, c=SEG)
                        if IS_F32R:
                            # round+mask in place; matmul reads the same tile
                            nc.vector.tensor_tensor(
                                rcast(w0f[:]).rearrange(
                                    "p (k s c) -> p k s c", k=2, c=SEG
                                ),
                                ap_in, msk, ALU.mult,
                            )
                            w0m_tiles.append(rcast(w0f[:]))
                        else:
                            w0m = w0_pool.tile([P, 1024], mm_dt, tag="w0m", name="w0m")
                            nc.vector.tensor_tensor(
                                w0m[:].rearrange("p (k s c) -> p k s c", k=2, c=SEG),
                                ap_in, msk, ALU.mult,
                            )
                            w0m_tiles.append(w0m[:])
                    for ht in range(4):  # h-tiles of 128 within the group
                        m = hg * 4 + ht
                        ps = ps0_pool.tile([P, T], F32, tag="ps0", name="ps0")
                        for k in range(D // P):
                            w0m = w0m_tiles[k // 2]
                            off = (k % 2) * 512 + ht * P
                            nc.tensor.matmul(
                                ps[:],
                                w0m[:, off : off + P],
                                xt_sb[k][:],
                                start=(k == 0),
                                stop=(k == D // P - 1),
                            )
                        nc.scalar.activation(
                            ht_sb[m][:], ps[:], AF.Relu, bias=bb0[:, m : m + 1]
                        )
                    # prefetch+mask two w1 pair-tiles per sub-group so the
                    # DMA queue alternates w0/w1 and L1 never waits
                    for pj in (2 * sub, 2 * sub + 1):
                        hc = pr * 8 + 2 * pj
                        w1f = w1f_pool.tile([P, 2048], F32, tag="w1f", name="w1f")
                        nc.sync.dma_start(
                            w1f[:].rearrange("p (k d) -> p k d", k=2),
                            w1T_d[hc * P : (hc + 2) * P, :].rearrange(
                                "(k p) d -> p k d", k=2
                            ),
                        )
                        seg_h = hc // 2
                        ap3f = w1f[:].rearrange("p (k d) -> p k d", k=2)
                        if IS_F32R:
                            ap3m = rcast(w1f[:]).rearrange("p (k d) -> p k d", k=2)
                            out_ap = rcast(w1f[:])
                        else:
                            w1m = w1_pool.tile([P, 2048], mm_dt, tag="w1m", name="w1m")
                            ap3m = w1m[:].rearrange("p (k d) -> p k d", k=2)
                            out_ap = w1m[:]
                        for jd, (c0, c1) in enumerate(
                            [(0, 512), (512, 768), (768, 1024)]
                        ):
                            nc.scalar.activation(
                                ap3m[:, :, c0:c1], ap3f[:, :, c0:c1], AF.Copy,
                                scale=gb[:, jd * 16 + seg_h : jd * 16 + seg_h + 1],
                            )
                        w1m_tiles.append(out_ap)

                # ---- L1 partial for this h-group pair (K = 8 x 128) ----
                for dt in range(D // P):  # 8 output d-tiles
                    ps = ps1_pool.tile([P, T], F32, tag="ps1", name="ps1")
                    for j in range(8):  # h-chunks across both groups
                        w1m = w1m_tiles[j // 2]
                        off = (j % 2) * 1024 + dt * P
                        nc.tensor.matmul(
                            ps[:],
                            w1m[:, off : off + P],
                            ht_sb[pr * 8 + j][:],
                            start=(j == 0),
                            stop=(j == 7),
                        )
                    if pr == 0:
                        nc.scalar.activation(
                            outacc[dt][:], ps[:], AF.Identity, bias=bb1[:, dt : dt + 1]
                        )
                    else:
                        nc.vector.tensor_tensor(
                            outacc[dt][:], ps[:], outacc[dt][:], ALU.add
                        )

            # ---------------- store ----------------
            for dt in range(D // P):
                nc.sync.dma_start(out_d[dt * P : (dt + 1) * P, :], outacc[dt][:])

    nc.compile()
    return nc


_NC_CACHE: dict[str, bass.Bass] = {}


def _get_nc() -> bass.Bass:
    if MM_DTYPE not in _NC_CACHE:
        _NC_CACHE[MM_DTYPE] = _build_nc()
    return _NC_CACHE[MM_DTYPE]


def make_in_maps(x, w0, b0, w1, b1, arch_embed, arch_mlp):
    """Host-side layout prep (pure reshape/transpose/tile, no arithmetic)."""
    w0T = np.ascontiguousarray(w0.T)                       # [D, H]
    w1T = np.ascontiguousarray(w1.T)                       # [H, D]
    b0r = np.ascontiguousarray(b0.reshape(H // P, P).T)    # [P, 32]
    b1r = np.ascontiguousarray(b1.reshape(D // P, P).T)    # [P, 8]
    ae9 = np.ascontiguousarray(np.repeat(arch_embed, 3).reshape(9, 1))
    am9 = np.ascontiguousarray(np.tile(arch_mlp, 3).reshape(9, 1))
    x3 = x.reshape(N_CORES, T, D)
    return [
        {
            "xT": np.ascontiguousarray(x3[c].T),           # [D, T]
            "w0T": w0T,
            "w1T": w1T,
            "b0r": b0r,
            "b1r": b1r,
            "ae9": ae9,
            "am9": am9,
            "k2": _K2,
        }
        for c in range(N_CORES)
    ]


def kernel(x, w0, b0, w1, b1, arch_embed, arch_mlp):
    x = np.asarray(x, dtype=np.float32)
    w0 = np.asarray(w0, dtype=np.float32)
    b0 = np.asarray(b0, dtype=np.float32)
    w1 = np.asarray(w1, dtype=np.float32)
    b1 = np.asarray(b1, dtype=np.float32)
    arch_embed = np.asarray(arch_embed, dtype=np.float32)
    arch_mlp = np.asarray(arch_mlp, dtype=np.float32)

    in_maps = make_in_maps(x, w0, b0, w1, b1, arch_embed, arch_mlp)
    nc = _get_nc()
    res = run_bass_kernel_spmd(nc, in_maps, core_ids=list(range(N_CORES)))
    out = np.stack([res.results[c]["outT"].T for c in range(N_CORES)], axis=0)
    return np.ascontiguousarray(out)  # [8, 512, 1024] float32
